# revision 9
# baseline (speedup 1.0000x reference)
"""DFA-GNN (max-aggregation message passing) Trainium2 kernel.

Problem (B=2, N=4096, E=65536, M=4, H=256), per batch b:
    coeff[e]  = edge_fts[b,e,:] @ edge_W + edge_b                  # [E]
    agg[n]    = max over edges e with tgt[e]==n of coeff[e] * hint[b, src[e]]
    out[b,n]  = (node_fts[b,n] + agg[n]) @ update_W + update_b     # [M,H] rows

Sharding: 8 cores = 2 batches x 4 target-node quarters (1024 nodes each).
Edges are bucketed by target node on the host (every node has exactly 16
incoming edges with this generator; general counts <=16 are padded by
duplicating an edge, which preserves the max).

v4 structure (bf16 data path, rel err ~4.7e-3 vs 2e-2 gate). Per block
(128 nodes, K=16 edge ranks):
  - 4x SWDGE dma_gather (512 descriptors of 2KB rows, round-robin over 4
    queues) -> gt tiles [128, 4*1024] bf16,
  - per-edge coeff via PE matmuls (eW stationary, host-transposed edge
    features streaming); ONE merged Act op adds edge_b and moves both
    PSUM rows [2(stride 64) x 1024] to SBUF; DMA spray to [128, 16] f32,
  - products: each rank k is an INDEPENDENT multiply (per-partition f32
    scalar t=coeff) into a slice of a wide tile P_h [128, 4096]: 3 ranks
    per tile on Act (~1055ns), 1 on DVE tensor_scalar (~1106ns),
  - max tree on WIDE tiles (DVE 2x amortizes the ~170ns init): for each
    pair of P tiles one [4096]-wide TT max (4 maxes in 2306ns), then
    folds [2048] + [1024], final join + node_fts add (TT, 825ns),
  - transpose to feature-major via DMA xbar transpose (sync HWDGE queue;
    frees PE of 8 transposes/block and Act of the xt copy),
  - update_W matmuls (8/block); update_b rank-1 matmuls only compiled
    when update_b != 0 host-side,
  - bf16 output, upcast to f32 on the host.

Engine model per block, calibrated on-device: DMA 6.1MB ~16.9us (HBM
bound, 22.3GB/s x 16 engines); Act 12 products + co_row + o copy
~15.1us; DVE 4 products + 15 tree maxes + nf add ~14.5us; PE coeff
8x~550ns + update 8x~500ns ~9us. DMA-bound -> ~135us + tails.

Measured dead ends kept so future sessions skip them: INT8 gather
(per-row scales folded into coeff) halves gather DMA to 68us and passes
accuracy (9.2e-3) but 1-byte operands disable DVE 2x/4x modes and slow
Act ~1.4x (int8 mul 1500ns, STT 1500ns, tensor_scalar 2464ns+) -- the
elementwise side becomes the bottleneck and the kernel REGRESSES to
229us (vs 183 baseline). fp8_e4m3 hint fails accuracy (3.0e-2);
fp8_e3m4 edge features (eW pre-scaled x64 to dodge subnormals) sim at
1.8e-2 -- too close to the gate to ship blind. DMA cannot read PSUM.
Act has no 2x mode ((N+352)/1.2GHz, dtype-independent claim is false
for int8). TT max is 2x_1p only (825ns/1024); STT is 1x-only.
Multi-index indirect DMA corrupts data; ap_gather is ~9x its cost
model; GpSimd can't run TT/STT and bulk elementwise there is
software-slow; in-place DVE/Act ops lose their perf mode; bf16 PSUM
cannot accumulate; PE matmul rejects int8 (fp8e3/e4/e5 ok); PE idles
down-clock (first matmuls after a gap run ~2x slow).
"""

import os
import sys

import numpy as np

for _p in ("/opt/trn_rl_repo", "/root/.axon_site/_ro/trn_rl_repo"):
    if os.path.isdir(_p) and _p not in sys.path:
        sys.path.insert(0, _p)

B, N, E, M, H = 2, 4096, 65536, 4, 256
MH = M * H            # 1024
P = 128               # partitions
K = 16                # edges per node (E // N)
NCORE = N // 4        # nodes per core (1024)
NB = NCORE // P       # node blocks per core (8)
EC = NCORE * K        # edges per core (16384)
ECB = P * K           # edges per block (2048)
KH = K // 4           # edge ranks per gather tile (4)
GT_BUFS = int(os.environ.get("KERNEL_GT_BUFS", "6"))
N_CORES = 8
N_SWDGE_Q = int(os.environ.get("KERNEL_SWDGE_Q", "4"))
# rank within each gather tile whose product runs on DVE (rest on Act)
DVE_RANK = int(os.environ.get("KERNEL_DVE_RANK", "0"))

_CACHE = {}

# Set by kernel() when KERNEL_TRACE=1: BassKernelResults of the last run.
LAST_RESULT = None


def _build(with_ub: bool):
    from concourse import bass, bacc, mybir, tile

    f32 = mybir.dt.float32
    i16 = mybir.dt.int16
    bf16 = mybir.dt.bfloat16

    nc = bacc.Bacc("TRN2", target_bir_lowering=False, debug=False,
                   num_devices=N_CORES, num_swdge_queues=N_SWDGE_Q)

    hint = nc.dram_tensor("hint", [N, MH], bf16, kind="ExternalInput")
    eftsT = nc.dram_tensor("eftsT", [H, EC], bf16, kind="ExternalInput")
    idx_d = nc.dram_tensor("idx16", [P, EC // 16], i16, kind="ExternalInput")
    nf_d = nc.dram_tensor("nf", [NCORE, MH], bf16, kind="ExternalInput")
    eW_d = nc.dram_tensor("eW", [P, 2], bf16, kind="ExternalInput")
    eb_d = nc.dram_tensor("eb", [P, 1], f32, kind="ExternalInput")
    uW_d = nc.dram_tensor("uW", [H, H], bf16, kind="ExternalInput")
    ub_d = nc.dram_tensor("ub", [1, H], bf16, kind="ExternalInput")
    out_d = nc.dram_tensor("out", [NCORE, MH], bf16, kind="ExternalOutput")

    with tile.TileContext(nc) as tc:
        from concourse.mybir import AluOpType as alu

        with (
            tc.tile_pool(name="const", bufs=1) as cpool,
            tc.tile_pool(name="efts", bufs=2) as epool,
            tc.tile_pool(name="gt", bufs=GT_BUFS) as gpool,
            tc.tile_pool(name="prod", bufs=1) as ppool,
            tc.tile_pool(name="tree", bufs=2) as tpool,
            tc.tile_pool(name="work", bufs=2) as wpool,
            tc.tile_pool(name="ps_coeff", bufs=2, space="PSUM") as ps_coeff,
            tc.tile_pool(name="ps_out", bufs=2, space="PSUM") as ps_out,
        ):
            # idx16 rides the Act-engine HWDGE queue: the sync queue floods
            # with edge features at startup and would delay the first
            # gather's index table.
            idx_t = cpool.tile([P, EC // 16], i16)
            nc.scalar.dma_start(out=idx_t[:], in_=idx_d[:])
            eW = cpool.tile([P, 2], bf16)
            nc.scalar.dma_start(out=eW[:], in_=eW_d[:])
            eb = cpool.tile([P, 1], f32)
            nc.scalar.dma_start(out=eb[:], in_=eb_d[:])
            uW0 = cpool.tile([P, H], bf16)
            uW1 = cpool.tile([P, H], bf16)
            nc.scalar.dma_start(out=uW0[:], in_=uW_d[0:P, :])
            nc.scalar.dma_start(out=uW1[:], in_=uW_d[P:2 * P, :])
            if with_ub:
                ub_row = cpool.tile([1, H], bf16)
                nc.scalar.dma_start(out=ub_row[:], in_=ub_d[:])
                ones1 = cpool.tile([1, P], bf16)
                nc.vector.memset(ones1[:], 1.0)

            coeffs = [None] * NB

            def emit_coeff(nb):
                # eftsT columns node-major in the block:
                # col nb*2048 + p*16 + k -> edge rank k of node p.
                efts0 = epool.tile([P, ECB], bf16, tag="efts0")
                efts1 = epool.tile([P, ECB], bf16, tag="efts1")
                nc.sync.dma_start(out=efts0[:],
                                  in_=eftsT[0:P, nb * ECB:(nb + 1) * ECB])
                nc.sync.dma_start(out=efts1[:],
                                  in_=eftsT[P:2 * P, nb * ECB:(nb + 1) * ECB])
                co_ps = ps_coeff.tile([128, 1024], f32, tag="co_ps",
                                      space="PSUM")
                for c in range(4):
                    pp, ff = (c % 2) * 64, (c // 2) * 512
                    nc.tensor.matmul(co_ps[pp:pp + 1, ff:ff + 512],
                                     lhsT=eW[:, 0:1],
                                     rhs=efts0[:, c * 512:(c + 1) * 512],
                                     start=True, stop=False)
                    nc.tensor.matmul(co_ps[pp:pp + 1, ff:ff + 512],
                                     lhsT=eW[:, 1:2],
                                     rhs=efts1[:, c * 512:(c + 1) * 512],
                                     start=False, stop=True)
                # one merged Act op: +edge_b and PSUM->SBUF for both rows
                # (partitions 0 and 64), then DMA spray to [128, 16].
                co_row = wpool.tile([P, 1024], f32, tag="co_row")
                for pp in (0, 64):
                    nc.scalar.add(co_row[pp:pp + 1, :], co_ps[pp:pp + 1, :],
                                  eb[pp:pp + 1, 0:1])
                coeff = wpool.tile([P, K], f32, tag="coeff")
                for c in range(4):
                    pp, ff = (c % 2) * 64, (c // 2) * 512
                    nc.sync.dma_start(
                        out=coeff[c * 32:(c + 1) * 32, :],
                        in_=co_row[pp:pp + 1, ff:ff + 512].rearrange(
                            "c (p k) -> c p k", k=K))
                coeffs[nb] = coeff

            emit_coeff(0)
            # one shared num_idxs register for every dma_gather (all 512)
            nidx_reg = nc.gpsimd.to_reg(P * KH)
            for nb in range(NB):
                # ---- gather: one SWDGE dma_gather per 4 edge ranks ----
                gts = []
                for h in range(K // KH):
                    gt = gpool.tile([P, KH * MH], bf16, tag="gt")
                    c0 = (nb * ECB + h * P * KH) // 16
                    nc.gpsimd.dma_gather(
                        gt[:].rearrange("p (g e) -> p g e", e=MH),
                        hint[:],
                        idx_t[:, c0:c0 + P * KH // 16],
                        P * KH, nidx_reg, MH,
                        queue_num=(nb * (K // KH) + h) % N_SWDGE_Q,
                    )
                    gts.append(gt)
                nf = wpool.tile([P, MH], bf16, tag="nf")
                nc.scalar.dma_start(out=nf[:], in_=nf_d[nb * P:(nb + 1) * P, :])
                if nb + 1 < NB:
                    emit_coeff(nb + 1)

                # ---- independent products into wide tiles ----
                t = coeffs[nb]
                ptiles = []
                for h in range(K // KH):
                    gt = gts[h]
                    pt = ppool.tile([P, KH * MH], bf16, tag=f"p{h}",
                                    name=f"p{h}")
                    for j in range(KH):
                        k = h * KH + j
                        src = gt[:, j * MH:(j + 1) * MH]
                        dst = pt[:, j * MH:(j + 1) * MH]
                        sc = t[:, k:k + 1]
                        if j == DVE_RANK:
                            nc.vector.tensor_scalar(
                                out=dst, in0=src, scalar1=sc,
                                scalar2=None, op0=alu.mult)
                        else:
                            nc.scalar.mul(dst, src, sc)
                    ptiles.append(pt)

                # ---- wide max tree: 15 maxes in 7 TT ops ----
                halves = []
                for h in range(2):
                    q = tpool.tile([P, KH * MH], bf16, tag=f"q{h}",
                                   name=f"q{h}")
                    nc.vector.tensor_tensor(out=q[:], in0=ptiles[2 * h][:],
                                            in1=ptiles[2 * h + 1][:],
                                            op=alu.max)
                    s2 = tpool.tile([P, 2 * MH], bf16, tag=f"s{h}",
                                    name=f"s{h}")
                    nc.vector.tensor_tensor(out=s2[:], in0=q[:, 0:2 * MH],
                                            in1=q[:, 2 * MH:4 * MH],
                                            op=alu.max)
                    a1 = tpool.tile([P, MH], bf16, tag=f"a{h}",
                                    name=f"a{h}")
                    nc.vector.tensor_tensor(out=a1[:], in0=s2[:, 0:MH],
                                            in1=s2[:, MH:2 * MH],
                                            op=alu.max)
                    halves.append(a1)

                comb = wpool.tile([P, MH], bf16, tag="comb")
                nc.vector.tensor_tensor(out=comb[:], in0=halves[0][:],
                                        in1=halves[1][:], op=alu.max)
                xf = wpool.tile([P, MH], bf16, tag="xf")
                nc.vector.tensor_tensor(out=xf[:], in0=comb[:], in1=nf[:],
                                        op=alu.add)
                xt = wpool.tile([P, MH], bf16, tag="xt")
                nc.sync.dma_start_transpose(
                    xt[:].rearrange("p (c n) -> p c n", n=P), xf[:])

                # ---- update matmuls ----
                o_ps = ps_out.tile([P, MH], f32, tag="o_ps", space="PSUM")
                for m in range(M):
                    nc.tensor.matmul(o_ps[:, m * H:(m + 1) * H],
                                     lhsT=xt[:, (2 * m) * P:(2 * m + 1) * P],
                                     rhs=uW0[:], start=True, stop=False)
                    nc.tensor.matmul(o_ps[:, m * H:(m + 1) * H],
                                     lhsT=xt[:, (2 * m + 1) * P:(2 * m + 2) * P],
                                     rhs=uW1[:], start=False,
                                     stop=not with_ub)
                    if with_ub:
                        nc.tensor.matmul(o_ps[:, m * H:(m + 1) * H],
                                         lhsT=ones1[0:1, :], rhs=ub_row[0:1, :],
                                         start=False, stop=True)
                o = wpool.tile([P, MH], bf16, tag="o")
                nc.scalar.copy(o[:], o_ps[:])
                nc.scalar.dma_start(out=out_d[nb * P:(nb + 1) * P, :], in_=o[:])

    nc.compile()
    return nc


def _install_ntff_hook():
    """Register the axon NTFF profiling hook if this image's antenv lacks it.

    Mirrors what trn_boot does when ``antenv.axon_hooks`` exists. Safe no-op
    on failure — tracing is skipped, execution still works.
    """
    import types

    try:
        import antenv.axon_hooks  # noqa: F401
        return
    except ImportError:
        pass
    try:
        import antenv
        from trn_agent_boot.trn_boot import _ntff_profile_via_ctypes

        hook = _ntff_profile_via_ctypes("/opt/axon/libaxon_pjrt.so")
        mod = types.ModuleType("antenv.axon_hooks")
        state = {"hook": hook}
        mod.get_axon_ntff_profile_hook = lambda: state["hook"]
        mod.set_axon_ntff_profile_hook = lambda h: state.update(hook=h)
        sys.modules["antenv.axon_hooks"] = mod
        antenv.axon_hooks = mod
    except Exception as e:  # pragma: no cover - best effort
        print(f"ntff hook install failed: {e}", file=sys.stderr)


def _edge_grid(tgt_b):
    """[N, K] edge ids bucketed by target node, padded by duplication."""
    counts = np.bincount(tgt_b, minlength=N)
    if counts.max() > K or counts.min() < 1:
        raise ValueError(f"edge counts per node outside [1, {K}]: "
                         f"min={counts.min()} max={counts.max()}")
    order = np.argsort(tgt_b, kind="stable")
    if (counts == K).all():
        return order.reshape(N, K)
    pos = np.zeros(N + 1, np.int64)
    np.cumsum(counts, out=pos[1:])
    offs = np.minimum(np.arange(K)[None, :], (counts - 1)[:, None])
    return order[pos[:-1, None] + offs]


def kernel(**inputs):
    global LAST_RESULT
    import ml_dtypes
    from concourse.bass_utils import run_bass_kernel_spmd

    wdt = ml_dtypes.bfloat16

    cfg = np.asarray(inputs["cfg_indices_padded"])
    hint_state = np.asarray(inputs["hint_state"], dtype=np.float32)
    node_fts = np.asarray(inputs["node_fts"], dtype=np.float32)
    edge_fts = np.asarray(inputs["edge_fts"], dtype=np.float32)
    edge_W = np.asarray(inputs["edge_W"], dtype=np.float32)
    edge_b = np.asarray(inputs["edge_b"], dtype=np.float32)
    update_W = np.asarray(inputs["update_W"], dtype=np.float32)
    update_b = np.asarray(inputs["update_b"], dtype=np.float32)

    src = np.asarray(cfg[..., 0], dtype=np.int64)
    tgt = np.asarray(cfg[..., 1], dtype=np.int64)

    with_ub = bool(np.any(update_b != 0.0))
    key = ("nc", with_ub)
    if key not in _CACHE:
        _CACHE[key] = _build(with_ub)
    nc = _CACHE[key]

    eW_in = np.ascontiguousarray(edge_W[:, 0].reshape(2, P).T).astype(wdt)
    eb_in = np.full((P, 1), edge_b[0], np.float32)
    ub_in = np.ascontiguousarray(update_b[None, :]).astype(wdt)
    uW_in = update_W.astype(wdt)

    in_maps = []
    for b in range(B):
        hint_b = np.ascontiguousarray(
            hint_state[b].reshape(N, MH)).astype(wdt)
        grid = _edge_grid(tgt[b])             # [N, K]
        srcg = src[b][grid]                   # [N, K]
        for q in range(4):
            g_q = grid[q * NCORE:(q + 1) * NCORE]    # [1024, K]
            s_q = srcg[q * NCORE:(q + 1) * NCORE]
            # gather index order: i = nb*2048 + k*128 + p, wrapped into
            # [16, EC/16] (idx16[r, c] = position c*16+r), tiled to 128 rows.
            gorder = s_q.reshape(NB, P, K).transpose(0, 2, 1)   # [nb, k, p]
            idx16 = np.ascontiguousarray(
                np.tile(gorder.reshape(EC // 16, 16).T, (8, 1))
            ).astype(np.int16)
            # edge-feature column order: j = nb*2048 + p*16 + k (node-major)
            eids = g_q.reshape(NB * P * K)
            efts_t = np.ascontiguousarray(edge_fts[b][eids].T).astype(wdt)
            nf_q = np.ascontiguousarray(
                node_fts[b, q * NCORE:(q + 1) * NCORE].reshape(NCORE, MH)
            ).astype(wdt)
            in_maps.append({
                "hint": hint_b,
                "eftsT": efts_t,
                "idx16": idx16,
                "nf": nf_q,
                "eW": eW_in,
                "eb": eb_in,
                "uW": uW_in,
                "ub": ub_in,
            })

    trace = bool(int(os.environ.get("KERNEL_TRACE", "0")))
    if trace:
        _install_ntff_hook()
    res = run_bass_kernel_spmd(nc, in_maps, core_ids=list(range(N_CORES)),
                               trace=trace)
    if trace:
        LAST_RESULT = res

    out = np.empty((B, N, M, H), np.float32)
    for b in range(B):
        for q in range(4):
            o = np.asarray(res.results[b * 4 + q]["out"], dtype=np.float32)
            out[b, q * NCORE:(q + 1) * NCORE] = o.reshape(NCORE, M, H)
    return out


# revision 11
# speedup vs baseline: 1.0133x; 1.0133x over previous
"""DFA-GNN (max-aggregation message passing) Trainium2 kernel.

Problem (B=2, N=4096, E=65536, M=4, H=256), per batch b:
    coeff[e]  = edge_fts[b,e,:] @ edge_W + edge_b                  # [E]
    agg[n]    = max over edges e with tgt[e]==n of coeff[e] * hint[b, src[e]]
    out[b,n]  = (node_fts[b,n] + agg[n]) @ update_W + update_b     # [M,H] rows

Sharding: 8 cores = 2 batches x 4 target-node quarters (1024 nodes each).
Edges are bucketed by target node on the host (every node has exactly 16
incoming edges with this generator; general counts <=16 are padded by
duplicating an edge, which preserves the max).

v4 structure (bf16 data path, rel err ~4.7e-3 vs 2e-2 gate). Per block
(128 nodes, K=16 edge ranks):
  - 4x SWDGE dma_gather (512 descriptors of 2KB rows, round-robin over 4
    queues) -> gt tiles [128, 4*1024] bf16,
  - per-edge coeff via PE matmuls (eW stationary, host-transposed edge
    features streaming); ONE merged Act op adds edge_b and moves both
    PSUM rows [2(stride 64) x 1024] to SBUF; DMA spray to [128, 16] f32,
  - products: each rank k is an INDEPENDENT multiply (per-partition f32
    scalar t=coeff) into a slice of a wide tile P_h [128, 4096]: 3 ranks
    per tile on Act (~1055ns), 1 on DVE tensor_scalar (~1106ns),
  - max tree on WIDE tiles (DVE 2x amortizes the ~170ns init): for each
    pair of P tiles one [4096]-wide TT max (4 maxes in 2306ns), then
    folds [2048] + [1024], final join + node_fts add (TT, 825ns),
  - transpose to feature-major via DMA xbar transpose (sync HWDGE queue;
    frees PE of 8 transposes/block and Act of the xt copy),
  - update_W matmuls (8/block); update_b rank-1 matmuls only compiled
    when update_b != 0 host-side,
  - bf16 output, upcast to f32 on the host.

Engine model per block, calibrated on-device: DMA 6.1MB ~16.9us (HBM
bound, 22.3GB/s x 16 engines); Act 12 products + co_row + o copy
~15.1us; DVE 4 products + 15 tree maxes + nf add ~14.5us; PE coeff
8x~550ns + update 8x~500ns ~9us. DMA-bound -> ~135us + tails.

Measured dead ends kept so future sessions skip them: INT8 gather
(per-row scales folded into coeff) halves gather DMA to 68us and passes
accuracy (9.2e-3) but 1-byte operands disable DVE 2x/4x modes and slow
Act ~1.4x (int8 mul 1500ns, STT 1500ns, tensor_scalar 2464ns+) -- the
elementwise side becomes the bottleneck and the kernel REGRESSES to
229us (vs 183 baseline). fp8_e4m3 hint fails accuracy (3.0e-2);
fp8_e3m4 edge features (eW pre-scaled x64 to dodge subnormals) sim at
1.8e-2 -- too close to the gate to ship blind. DMA cannot read PSUM.
Act has no 2x mode ((N+352)/1.2GHz, dtype-independent claim is false
for int8). TT max is 2x_1p only (825ns/1024); STT is 1x-only.
Multi-index indirect DMA corrupts data; ap_gather is ~9x its cost
model; GpSimd can't run TT/STT and bulk elementwise there is
software-slow; in-place DVE/Act ops lose their perf mode; bf16 PSUM
cannot accumulate; PE matmul rejects int8 (fp8e3/e4/e5 ok); PE idles
down-clock (first matmuls after a gap run ~2x slow).
"""

import os
import sys

import numpy as np

for _p in ("/opt/trn_rl_repo", "/root/.axon_site/_ro/trn_rl_repo"):
    if os.path.isdir(_p) and _p not in sys.path:
        sys.path.insert(0, _p)

B, N, E, M, H = 2, 4096, 65536, 4, 256
MH = M * H            # 1024
P = 128               # partitions
K = 16                # edges per node (E // N)
NCORE = N // 4        # nodes per core (1024)
NB = NCORE // P       # node blocks per core (8)
EC = NCORE * K        # edges per core (16384)
ECB = P * K           # edges per block (2048)
KH = K // 4           # edge ranks per gather tile (4)
GT_BUFS = int(os.environ.get("KERNEL_GT_BUFS", "6"))
N_CORES = 8
N_SWDGE_Q = int(os.environ.get("KERNEL_SWDGE_Q", "4"))
# rank within each gather tile whose product runs on DVE (rest on Act)
DVE_RANK = int(os.environ.get("KERNEL_DVE_RANK", "0"))

_CACHE = {}

# Set by kernel() when KERNEL_TRACE=1: BassKernelResults of the last run.
LAST_RESULT = None


def _build(with_ub: bool):
    from concourse import bass, bacc, mybir, tile

    f32 = mybir.dt.float32
    i16 = mybir.dt.int16
    bf16 = mybir.dt.bfloat16

    nc = bacc.Bacc("TRN2", target_bir_lowering=False, debug=False,
                   num_devices=N_CORES, num_swdge_queues=N_SWDGE_Q)

    hint = nc.dram_tensor("hint", [N, MH], bf16, kind="ExternalInput")
    eftsT = nc.dram_tensor("eftsT", [H, EC], bf16, kind="ExternalInput")
    idx_d = nc.dram_tensor("idx16", [P, EC // 16], i16, kind="ExternalInput")
    nf_d = nc.dram_tensor("nf", [NCORE, MH], bf16, kind="ExternalInput")
    eW_d = nc.dram_tensor("eW", [P, 2], bf16, kind="ExternalInput")
    eb_d = nc.dram_tensor("eb", [P, 1], f32, kind="ExternalInput")
    uW_d = nc.dram_tensor("uW", [H, H], bf16, kind="ExternalInput")
    ub_d = nc.dram_tensor("ub", [1, H], bf16, kind="ExternalInput")
    out_d = nc.dram_tensor("out", [NCORE, MH], bf16, kind="ExternalOutput")

    with tile.TileContext(nc) as tc:
        from concourse.mybir import AluOpType as alu

        with (
            tc.tile_pool(name="const", bufs=1) as cpool,
            tc.tile_pool(name="efts", bufs=2) as epool,
            tc.tile_pool(name="gt", bufs=GT_BUFS) as gpool,
            tc.tile_pool(name="prod", bufs=2) as ppool,
            tc.tile_pool(name="tree", bufs=2) as tpool,
            tc.tile_pool(name="treeq", bufs=1) as tqpool,
            tc.tile_pool(name="corow", bufs=1) as crpool,
            tc.tile_pool(name="work", bufs=2) as wpool,
            tc.tile_pool(name="ps_coeff", bufs=2, space="PSUM") as ps_coeff,
            tc.tile_pool(name="ps_out", bufs=2, space="PSUM") as ps_out,
        ):
            # idx16 rides the Act-engine HWDGE queue: the sync queue floods
            # with edge features at startup and would delay the first
            # gather's index table.
            idx_t = cpool.tile([P, EC // 16], i16)
            nc.scalar.dma_start(out=idx_t[:], in_=idx_d[:])
            eW = cpool.tile([P, 2], bf16)
            nc.scalar.dma_start(out=eW[:], in_=eW_d[:])
            eb = cpool.tile([P, 1], f32)
            nc.scalar.dma_start(out=eb[:], in_=eb_d[:])
            uW0 = cpool.tile([P, H], bf16)
            uW1 = cpool.tile([P, H], bf16)
            nc.scalar.dma_start(out=uW0[:], in_=uW_d[0:P, :])
            nc.scalar.dma_start(out=uW1[:], in_=uW_d[P:2 * P, :])
            if with_ub:
                ub_row = cpool.tile([1, H], bf16)
                nc.scalar.dma_start(out=ub_row[:], in_=ub_d[:])
                ones1 = cpool.tile([1, P], bf16)
                nc.vector.memset(ones1[:], 1.0)

            coeffs = [None] * NB

            def emit_coeff(nb):
                # eftsT columns node-major in the block:
                # col nb*2048 + p*16 + k -> edge rank k of node p.
                efts0 = epool.tile([P, ECB], bf16, tag="efts0")
                efts1 = epool.tile([P, ECB], bf16, tag="efts1")
                nc.sync.dma_start(out=efts0[:],
                                  in_=eftsT[0:P, nb * ECB:(nb + 1) * ECB])
                nc.sync.dma_start(out=efts1[:],
                                  in_=eftsT[P:2 * P, nb * ECB:(nb + 1) * ECB])
                co_ps = ps_coeff.tile([128, 1024], f32, tag="co_ps",
                                      space="PSUM")
                for c in range(4):
                    pp, ff = (c % 2) * 64, (c // 2) * 512
                    nc.tensor.matmul(co_ps[pp:pp + 1, ff:ff + 512],
                                     lhsT=eW[:, 0:1],
                                     rhs=efts0[:, c * 512:(c + 1) * 512],
                                     start=True, stop=False)
                    nc.tensor.matmul(co_ps[pp:pp + 1, ff:ff + 512],
                                     lhsT=eW[:, 1:2],
                                     rhs=efts1[:, c * 512:(c + 1) * 512],
                                     start=False, stop=True)
                # one merged Act op: +edge_b and PSUM->SBUF for both rows
                # (partitions 0 and 64), then DMA spray to [128, 16].
                co_row = crpool.tile([P, 1024], f32, tag="co_row")
                for pp in (0, 64):
                    nc.scalar.add(co_row[pp:pp + 1, :], co_ps[pp:pp + 1, :],
                                  eb[pp:pp + 1, 0:1])
                coeff = wpool.tile([P, K], f32, tag="coeff")
                for c in range(4):
                    pp, ff = (c % 2) * 64, (c // 2) * 512
                    nc.sync.dma_start(
                        out=coeff[c * 32:(c + 1) * 32, :],
                        in_=co_row[pp:pp + 1, ff:ff + 512].rearrange(
                            "c (p k) -> c p k", k=K))
                coeffs[nb] = coeff

            emit_coeff(0)
            # one shared num_idxs register for every dma_gather (all 512)
            nidx_reg = nc.gpsimd.to_reg(P * KH)
            for nb in range(NB):
                # ---- gather: one SWDGE dma_gather per 4 edge ranks ----
                gts = []
                for h in range(K // KH):
                    gt = gpool.tile([P, KH * MH], bf16, tag="gt")
                    c0 = (nb * ECB + h * P * KH) // 16
                    nc.gpsimd.dma_gather(
                        gt[:].rearrange("p (g e) -> p g e", e=MH),
                        hint[:],
                        idx_t[:, c0:c0 + P * KH // 16],
                        P * KH, nidx_reg, MH,
                        queue_num=(nb * (K // KH) + h) % N_SWDGE_Q,
                    )
                    gts.append(gt)
                nf = wpool.tile([P, MH], bf16, tag="nf")
                nc.scalar.dma_start(out=nf[:], in_=nf_d[nb * P:(nb + 1) * P, :])
                if nb + 1 < NB:
                    emit_coeff(nb + 1)

                # ---- independent products into wide tiles ----
                t = coeffs[nb]
                ptiles = []
                for h in range(K // KH):
                    gt = gts[h]
                    pt = ppool.tile([P, KH * MH], bf16, tag=f"p{h}",
                                    name=f"p{h}")
                    for j in range(KH):
                        k = h * KH + j
                        src = gt[:, j * MH:(j + 1) * MH]
                        dst = pt[:, j * MH:(j + 1) * MH]
                        sc = t[:, k:k + 1]
                        if j in (0, 2):
                            nc.vector.tensor_scalar(
                                out=dst, in0=src, scalar1=sc,
                                scalar2=None, op0=alu.mult)
                        else:
                            nc.scalar.mul(dst, src, sc)
                    ptiles.append(pt)

                # ---- wide max tree: 15 maxes in 7 TT ops ----
                halves = []
                for h in range(2):
                    q = tqpool.tile([P, KH * MH], bf16, tag=f"q{h}",
                                    name=f"q{h}")
                    nc.vector.tensor_tensor(out=q[:], in0=ptiles[2 * h][:],
                                            in1=ptiles[2 * h + 1][:],
                                            op=alu.max)
                    s2 = tpool.tile([P, 2 * MH], bf16, tag=f"s{h}",
                                    name=f"s{h}")
                    nc.vector.tensor_tensor(out=s2[:], in0=q[:, 0:2 * MH],
                                            in1=q[:, 2 * MH:4 * MH],
                                            op=alu.max)
                    a1 = tpool.tile([P, MH], bf16, tag=f"a{h}",
                                    name=f"a{h}")
                    nc.vector.tensor_tensor(out=a1[:], in0=s2[:, 0:MH],
                                            in1=s2[:, MH:2 * MH],
                                            op=alu.max)
                    halves.append(a1)

                comb = wpool.tile([P, MH], bf16, tag="comb")
                nc.vector.tensor_tensor(out=comb[:], in0=halves[0][:],
                                        in1=halves[1][:], op=alu.max)
                xf = wpool.tile([P, MH], bf16, tag="xf")
                nc.vector.tensor_tensor(out=xf[:], in0=comb[:], in1=nf[:],
                                        op=alu.add)
                xt = wpool.tile([P, MH], bf16, tag="xt")
                nc.sync.dma_start_transpose(
                    xt[:].rearrange("p (c n) -> p c n", n=P), xf[:])

                # ---- update matmuls ----
                o_ps = ps_out.tile([P, MH], f32, tag="o_ps", space="PSUM")
                for m in range(M):
                    nc.tensor.matmul(o_ps[:, m * H:(m + 1) * H],
                                     lhsT=xt[:, (2 * m) * P:(2 * m + 1) * P],
                                     rhs=uW0[:], start=True, stop=False)
                    nc.tensor.matmul(o_ps[:, m * H:(m + 1) * H],
                                     lhsT=xt[:, (2 * m + 1) * P:(2 * m + 2) * P],
                                     rhs=uW1[:], start=False,
                                     stop=not with_ub)
                    if with_ub:
                        nc.tensor.matmul(o_ps[:, m * H:(m + 1) * H],
                                         lhsT=ones1[0:1, :], rhs=ub_row[0:1, :],
                                         start=False, stop=True)
                o = wpool.tile([P, MH], bf16, tag="o")
                nc.scalar.copy(o[:], o_ps[:])
                nc.scalar.dma_start(out=out_d[nb * P:(nb + 1) * P, :], in_=o[:])

    nc.compile()
    return nc


def _install_ntff_hook():
    """Register the axon NTFF profiling hook if this image's antenv lacks it.

    Mirrors what trn_boot does when ``antenv.axon_hooks`` exists. Safe no-op
    on failure — tracing is skipped, execution still works.
    """
    import types

    try:
        import antenv.axon_hooks  # noqa: F401
        return
    except ImportError:
        pass
    try:
        import antenv
        from trn_agent_boot.trn_boot import _ntff_profile_via_ctypes

        hook = _ntff_profile_via_ctypes("/opt/axon/libaxon_pjrt.so")
        mod = types.ModuleType("antenv.axon_hooks")
        state = {"hook": hook}
        mod.get_axon_ntff_profile_hook = lambda: state["hook"]
        mod.set_axon_ntff_profile_hook = lambda h: state.update(hook=h)
        sys.modules["antenv.axon_hooks"] = mod
        antenv.axon_hooks = mod
    except Exception as e:  # pragma: no cover - best effort
        print(f"ntff hook install failed: {e}", file=sys.stderr)


def _edge_grid(tgt_b):
    """[N, K] edge ids bucketed by target node, padded by duplication."""
    counts = np.bincount(tgt_b, minlength=N)
    if counts.max() > K or counts.min() < 1:
        raise ValueError(f"edge counts per node outside [1, {K}]: "
                         f"min={counts.min()} max={counts.max()}")
    order = np.argsort(tgt_b, kind="stable")
    if (counts == K).all():
        return order.reshape(N, K)
    pos = np.zeros(N + 1, np.int64)
    np.cumsum(counts, out=pos[1:])
    offs = np.minimum(np.arange(K)[None, :], (counts - 1)[:, None])
    return order[pos[:-1, None] + offs]


def kernel(**inputs):
    global LAST_RESULT
    import ml_dtypes
    from concourse.bass_utils import run_bass_kernel_spmd

    wdt = ml_dtypes.bfloat16

    cfg = np.asarray(inputs["cfg_indices_padded"])
    hint_state = np.asarray(inputs["hint_state"], dtype=np.float32)
    node_fts = np.asarray(inputs["node_fts"], dtype=np.float32)
    edge_fts = np.asarray(inputs["edge_fts"], dtype=np.float32)
    edge_W = np.asarray(inputs["edge_W"], dtype=np.float32)
    edge_b = np.asarray(inputs["edge_b"], dtype=np.float32)
    update_W = np.asarray(inputs["update_W"], dtype=np.float32)
    update_b = np.asarray(inputs["update_b"], dtype=np.float32)

    src = np.asarray(cfg[..., 0], dtype=np.int64)
    tgt = np.asarray(cfg[..., 1], dtype=np.int64)

    with_ub = bool(np.any(update_b != 0.0))
    key = ("nc", with_ub)
    if key not in _CACHE:
        _CACHE[key] = _build(with_ub)
    nc = _CACHE[key]

    eW_in = np.ascontiguousarray(edge_W[:, 0].reshape(2, P).T).astype(wdt)
    eb_in = np.full((P, 1), edge_b[0], np.float32)
    ub_in = np.ascontiguousarray(update_b[None, :]).astype(wdt)
    uW_in = update_W.astype(wdt)

    in_maps = []
    for b in range(B):
        hint_b = np.ascontiguousarray(
            hint_state[b].reshape(N, MH)).astype(wdt)
        grid = _edge_grid(tgt[b])             # [N, K]
        srcg = src[b][grid]                   # [N, K]
        for q in range(4):
            g_q = grid[q * NCORE:(q + 1) * NCORE]    # [1024, K]
            s_q = srcg[q * NCORE:(q + 1) * NCORE]
            # gather index order: i = nb*2048 + k*128 + p, wrapped into
            # [16, EC/16] (idx16[r, c] = position c*16+r), tiled to 128 rows.
            gorder = s_q.reshape(NB, P, K).transpose(0, 2, 1)   # [nb, k, p]
            idx16 = np.ascontiguousarray(
                np.tile(gorder.reshape(EC // 16, 16).T, (8, 1))
            ).astype(np.int16)
            # edge-feature column order: j = nb*2048 + p*16 + k (node-major)
            eids = g_q.reshape(NB * P * K)
            efts_t = np.ascontiguousarray(edge_fts[b][eids].T).astype(wdt)
            nf_q = np.ascontiguousarray(
                node_fts[b, q * NCORE:(q + 1) * NCORE].reshape(NCORE, MH)
            ).astype(wdt)
            in_maps.append({
                "hint": hint_b,
                "eftsT": efts_t,
                "idx16": idx16,
                "nf": nf_q,
                "eW": eW_in,
                "eb": eb_in,
                "uW": uW_in,
                "ub": ub_in,
            })

    trace = bool(int(os.environ.get("KERNEL_TRACE", "0")))
    if trace:
        _install_ntff_hook()
    res = run_bass_kernel_spmd(nc, in_maps, core_ids=list(range(N_CORES)),
                               trace=trace)
    if trace:
        LAST_RESULT = res

    out = np.empty((B, N, M, H), np.float32)
    for b in range(B):
        for q in range(4):
            o = np.asarray(res.results[b * 4 + q]["out"], dtype=np.float32)
            out[b, q * NCORE:(q + 1) * NCORE] = o.reshape(NCORE, M, H)
    return out


# revision 12
# speedup vs baseline: 1.1552x; 1.1401x over previous
"""DFA-GNN (max-aggregation message passing) Trainium2 kernel.

Problem (B=2, N=4096, E=65536, M=4, H=256), per batch b:
    coeff[e]  = edge_fts[b,e,:] @ edge_W + edge_b                  # [E]
    agg[n]    = max over edges e with tgt[e]==n of coeff[e] * hint[b, src[e]]
    out[b,n]  = (node_fts[b,n] + agg[n]) @ update_W + update_b     # [M,H] rows

Sharding: 8 cores = 2 batches x 4 target-node quarters (1024 nodes each).
Edges are bucketed by target node on the host (every node has exactly 16
incoming edges with this generator; general counts <=16 are padded by
duplicating an edge, which preserves the max).

v4 structure (bf16 data path, rel err ~4.7e-3 vs 2e-2 gate). Per block
(128 nodes, K=16 edge ranks):
  - 4x SWDGE dma_gather (512 descriptors of 2KB rows, round-robin over 4
    queues) -> gt tiles [128, 4*1024] bf16,
  - per-edge coeff via PE matmuls (eW stationary, host-transposed edge
    features streaming); ONE merged Act op adds edge_b and moves both
    PSUM rows [2(stride 64) x 1024] to SBUF; DMA spray to [128, 16] f32,
  - products: each rank k is an INDEPENDENT multiply (per-partition f32
    scalar t=coeff) into a slice of a wide tile P_h [128, 4096]: 3 ranks
    per tile on Act (~1055ns), 1 on DVE tensor_scalar (~1106ns),
  - max tree on WIDE tiles (DVE 2x amortizes the ~170ns init): for each
    pair of P tiles one [4096]-wide TT max (4 maxes in 2306ns), then
    folds [2048] + [1024], final join + node_fts add (TT, 825ns),
  - transpose to feature-major via DMA xbar transpose (sync HWDGE queue;
    frees PE of 8 transposes/block and Act of the xt copy),
  - update_W matmuls (8/block); update_b rank-1 matmuls only compiled
    when update_b != 0 host-side,
  - bf16 output, upcast to f32 on the host.

Engine model per block, calibrated on-device: DMA 6.1MB ~16.9us (HBM
bound, 22.3GB/s x 16 engines); Act 12 products + co_row + o copy
~15.1us; DVE 4 products + 15 tree maxes + nf add ~14.5us; PE coeff
8x~550ns + update 8x~500ns ~9us. DMA-bound -> ~135us + tails.

Measured dead ends kept so future sessions skip them: INT8 gather
(per-row scales folded into coeff) halves gather DMA to 68us and passes
accuracy (9.2e-3) but 1-byte operands disable DVE 2x/4x modes and slow
Act ~1.4x (int8 mul 1500ns, STT 1500ns, tensor_scalar 2464ns+) -- the
elementwise side becomes the bottleneck and the kernel REGRESSES to
229us (vs 183 baseline). fp8_e4m3 hint fails accuracy (3.0e-2);
fp8_e3m4 edge features (eW pre-scaled x64 to dodge subnormals) sim at
1.8e-2 -- too close to the gate to ship blind. DMA cannot read PSUM.
Act has no 2x mode ((N+352)/1.2GHz, dtype-independent claim is false
for int8). TT max is 2x_1p only (825ns/1024); STT is 1x-only.
Multi-index indirect DMA corrupts data; ap_gather is ~9x its cost
model; GpSimd can't run TT/STT and bulk elementwise there is
software-slow; in-place DVE/Act ops lose their perf mode; bf16 PSUM
cannot accumulate; PE matmul rejects int8 (fp8e3/e4/e5 ok); PE idles
down-clock (first matmuls after a gap run ~2x slow).
"""

import os
import sys

import numpy as np

for _p in ("/opt/trn_rl_repo", "/root/.axon_site/_ro/trn_rl_repo"):
    if os.path.isdir(_p) and _p not in sys.path:
        sys.path.insert(0, _p)

B, N, E, M, H = 2, 4096, 65536, 4, 256
MH = M * H            # 1024
P = 128               # partitions
K = 16                # edges per node (E // N)
NCORE = N // 4        # nodes per core (1024)
NB = NCORE // P       # node blocks per core (8)
EC = NCORE * K        # edges per core (16384)
ECB = P * K           # edges per block (2048)
KH = K // 4           # edge ranks per gather tile (4)
GT_BUFS = int(os.environ.get("KERNEL_GT_BUFS", "6"))
N_CORES = 8
N_SWDGE_Q = int(os.environ.get("KERNEL_SWDGE_Q", "4"))
# rank within each gather tile whose product runs on DVE (rest on Act)
DVE_RANK = int(os.environ.get("KERNEL_DVE_RANK", "0"))

_CACHE = {}

# Set by kernel() when KERNEL_TRACE=1: BassKernelResults of the last run.
LAST_RESULT = None


def _build(with_ub: bool):
    from concourse import bass, bacc, mybir, tile

    f32 = mybir.dt.float32
    i16 = mybir.dt.int16
    bf16 = mybir.dt.bfloat16

    nc = bacc.Bacc("TRN2", target_bir_lowering=False, debug=False,
                   num_devices=N_CORES, num_swdge_queues=N_SWDGE_Q)

    hint = nc.dram_tensor("hint", [N, MH], bf16, kind="ExternalInput")
    eftsT = nc.dram_tensor("eftsT", [H, EC], bf16, kind="ExternalInput")
    idx_d = nc.dram_tensor("idx16", [P, EC // 16], i16, kind="ExternalInput")
    nf_d = nc.dram_tensor("nf", [NCORE, MH], bf16, kind="ExternalInput")
    eW_d = nc.dram_tensor("eW", [P, 2], bf16, kind="ExternalInput")
    eb_d = nc.dram_tensor("eb", [P, 1], f32, kind="ExternalInput")
    uW_d = nc.dram_tensor("uW", [H, H], bf16, kind="ExternalInput")
    ub_d = nc.dram_tensor("ub", [1, H], bf16, kind="ExternalInput")
    out_d = nc.dram_tensor("out", [NCORE, MH], bf16, kind="ExternalOutput")

    with tile.TileContext(nc) as tc:
        from concourse.mybir import AluOpType as alu

        with (
            tc.tile_pool(name="const", bufs=1) as cpool,
            tc.tile_pool(name="efts", bufs=2) as epool,
            tc.tile_pool(name="gt", bufs=GT_BUFS) as gpool,
            tc.tile_pool(name="prod", bufs=2) as ppool,
            tc.tile_pool(name="tree", bufs=2) as tpool,
            tc.tile_pool(name="treeq", bufs=1) as tqpool,
            tc.tile_pool(name="corow", bufs=1) as crpool,
            tc.tile_pool(name="work", bufs=2) as wpool,
            tc.tile_pool(name="ps_coeff", bufs=2, space="PSUM") as ps_coeff,
            tc.tile_pool(name="ps_out", bufs=2, space="PSUM") as ps_out,
        ):
            # idx16 rides the Act-engine HWDGE queue: the sync queue floods
            # with edge features at startup and would delay the first
            # gather's index table.
            idx_t = cpool.tile([P, EC // 16], i16)
            nc.scalar.dma_start(out=idx_t[:], in_=idx_d[:])
            eW = cpool.tile([P, 2], bf16)
            nc.scalar.dma_start(out=eW[:], in_=eW_d[:])
            eb = cpool.tile([P, 1], f32)
            nc.scalar.dma_start(out=eb[:], in_=eb_d[:])
            uW0 = cpool.tile([P, H], bf16)
            uW1 = cpool.tile([P, H], bf16)
            nc.scalar.dma_start(out=uW0[:], in_=uW_d[0:P, :])
            nc.scalar.dma_start(out=uW1[:], in_=uW_d[P:2 * P, :])
            if with_ub:
                ub_row = cpool.tile([1, H], bf16)
                nc.scalar.dma_start(out=ub_row[:], in_=ub_d[:])
                ones1 = cpool.tile([1, P], bf16)
                nc.vector.memset(ones1[:], 1.0)

            coeffs = [None] * NB
            co_pss = [None] * NB

            def emit_coeff_mm(nb):
                # eftsT columns node-major in the block:
                # col nb*2048 + p*16 + k -> edge rank k of node p.
                efts0 = epool.tile([P, ECB], bf16, tag="efts0")
                efts1 = epool.tile([P, ECB], bf16, tag="efts1")
                nc.sync.dma_start(out=efts0[:],
                                  in_=eftsT[0:P, nb * ECB:(nb + 1) * ECB])
                nc.sync.dma_start(out=efts1[:],
                                  in_=eftsT[P:2 * P, nb * ECB:(nb + 1) * ECB])
                co_ps = ps_coeff.tile([128, 1024], f32, tag="co_ps",
                                      space="PSUM")
                for c in range(4):
                    pp, ff = (c % 2) * 64, (c // 2) * 512
                    nc.tensor.matmul(co_ps[pp:pp + 1, ff:ff + 512],
                                     lhsT=eW[:, 0:1],
                                     rhs=efts0[:, c * 512:(c + 1) * 512],
                                     start=True, stop=False)
                    nc.tensor.matmul(co_ps[pp:pp + 1, ff:ff + 512],
                                     lhsT=eW[:, 1:2],
                                     rhs=efts1[:, c * 512:(c + 1) * 512],
                                     start=False, stop=True)
                co_pss[nb] = co_ps

            def emit_coeff_fix(nb):
                # merged Act ops: +edge_b and PSUM->SBUF (rows at partitions
                # 0 and 64), then DMA spray to the [128, 16] chain layout.
                co_ps = co_pss[nb]
                co_row = crpool.tile([P, 1024], f32, tag="co_row")
                for pp in (0, 64):
                    nc.scalar.add(co_row[pp:pp + 1, :], co_ps[pp:pp + 1, :],
                                  eb[pp:pp + 1, 0:1])
                coeff = wpool.tile([P, K], f32, tag="coeff")
                for c in range(4):
                    pp, ff = (c % 2) * 64, (c // 2) * 512
                    nc.sync.dma_start(
                        out=coeff[c * 32:(c + 1) * 32, :],
                        in_=co_row[pp:pp + 1, ff:ff + 512].rearrange(
                            "c (p k) -> c p k", k=K))
                coeffs[nb] = coeff

            emit_coeff_mm(0)
            emit_coeff_fix(0)
            pending_out = [None]
            # one shared num_idxs register for every dma_gather (all 512)
            nidx_reg = nc.gpsimd.to_reg(P * KH)
            for nb in range(NB):
                # ---- gather: one SWDGE dma_gather per 4 edge ranks ----
                gts = []
                for h in range(K // KH):
                    gt = gpool.tile([P, KH * MH], bf16, tag="gt")
                    c0 = (nb * ECB + h * P * KH) // 16
                    nc.gpsimd.dma_gather(
                        gt[:].rearrange("p (g e) -> p g e", e=MH),
                        hint[:],
                        idx_t[:, c0:c0 + P * KH // 16],
                        P * KH, nidx_reg, MH,
                        queue_num=(nb * (K // KH) + h) % N_SWDGE_Q,
                    )
                    gts.append(gt)
                nf = wpool.tile([P, MH], bf16, tag="nf")
                nc.scalar.dma_start(out=nf[:], in_=nf_d[nb * P:(nb + 1) * P, :])
                if nb + 1 < NB:
                    emit_coeff_mm(nb + 1)

                # ---- products into wide tiles, interleaved with the max
                # tree so neither engine queue blocks on the last gather ----
                t = coeffs[nb]
                halves = []

                def emit_products(h):
                    gt = gts[h]
                    pt = ppool.tile([P, KH * MH], bf16, tag=f"p{h}",
                                    name=f"p{h}")
                    for j in range(KH):
                        k = h * KH + j
                        src = gt[:, j * MH:(j + 1) * MH]
                        dst = pt[:, j * MH:(j + 1) * MH]
                        sc = t[:, k:k + 1]
                        if j in (0, 2):
                            nc.vector.tensor_scalar(
                                out=dst, in0=src, scalar1=sc,
                                scalar2=None, op0=alu.mult)
                        else:
                            nc.scalar.mul(dst, src, sc)
                    return pt

                def emit_tree_half(h, pt0, pt1):
                    q = tqpool.tile([P, KH * MH], bf16, tag=f"q{h}",
                                    name=f"q{h}")
                    nc.vector.tensor_tensor(out=q[:], in0=pt0[:], in1=pt1[:],
                                            op=alu.max)
                    s2 = tpool.tile([P, 2 * MH], bf16, tag=f"s{h}",
                                    name=f"s{h}")
                    nc.vector.tensor_tensor(out=s2[:], in0=q[:, 0:2 * MH],
                                            in1=q[:, 2 * MH:4 * MH],
                                            op=alu.max)
                    a1 = tpool.tile([P, MH], bf16, tag=f"a{h}",
                                    name=f"a{h}")
                    nc.vector.tensor_tensor(out=a1[:], in0=s2[:, 0:MH],
                                            in1=s2[:, MH:2 * MH],
                                            op=alu.max)
                    halves.append(a1)

                pt0 = emit_products(0)
                pt1 = emit_products(1)
                emit_tree_half(0, pt0, pt1)
                pt2 = emit_products(2)
                pt3 = emit_products(3)
                # flush the previous block's output while this block's last
                # products run, and fix up the next block's coefficients --
                # emitting either earlier would head-of-line block the Act
                # queue behind data that is not ready yet.
                if pending_out[0] is not None:
                    po_ps, pnb = pending_out[0]
                    o = wpool.tile([P, MH], bf16, tag="o")
                    nc.scalar.copy(o[:], po_ps[:])
                    nc.scalar.dma_start(out=out_d[pnb * P:(pnb + 1) * P, :],
                                        in_=o[:])
                if nb + 1 < NB:
                    emit_coeff_fix(nb + 1)
                emit_tree_half(1, pt2, pt3)

                comb = wpool.tile([P, MH], bf16, tag="comb")
                nc.vector.tensor_tensor(out=comb[:], in0=halves[0][:],
                                        in1=halves[1][:], op=alu.max)
                xf = wpool.tile([P, MH], bf16, tag="xf")
                nc.vector.tensor_tensor(out=xf[:], in0=comb[:], in1=nf[:],
                                        op=alu.add)
                xt = wpool.tile([P, MH], bf16, tag="xt")
                nc.sync.dma_start_transpose(
                    xt[:].rearrange("p (c n) -> p c n", n=P), xf[:])

                # ---- update matmuls ----
                o_ps = ps_out.tile([P, MH], f32, tag="o_ps", space="PSUM")
                for m in range(M):
                    nc.tensor.matmul(o_ps[:, m * H:(m + 1) * H],
                                     lhsT=xt[:, (2 * m) * P:(2 * m + 1) * P],
                                     rhs=uW0[:], start=True, stop=False)
                    nc.tensor.matmul(o_ps[:, m * H:(m + 1) * H],
                                     lhsT=xt[:, (2 * m + 1) * P:(2 * m + 2) * P],
                                     rhs=uW1[:], start=False,
                                     stop=not with_ub)
                    if with_ub:
                        nc.tensor.matmul(o_ps[:, m * H:(m + 1) * H],
                                         lhsT=ones1[0:1, :], rhs=ub_row[0:1, :],
                                         start=False, stop=True)
                pending_out[0] = (o_ps, nb)

            po_ps, pnb = pending_out[0]
            o_last = wpool.tile([P, MH], bf16, tag="o")
            nc.scalar.copy(o_last[:], po_ps[:])
            nc.scalar.dma_start(out=out_d[pnb * P:(pnb + 1) * P, :],
                                in_=o_last[:])

    nc.compile()
    return nc


def _install_ntff_hook():
    """Register the axon NTFF profiling hook if this image's antenv lacks it.

    Mirrors what trn_boot does when ``antenv.axon_hooks`` exists. Safe no-op
    on failure — tracing is skipped, execution still works.
    """
    import types

    try:
        import antenv.axon_hooks  # noqa: F401
        return
    except ImportError:
        pass
    try:
        import antenv
        from trn_agent_boot.trn_boot import _ntff_profile_via_ctypes

        hook = _ntff_profile_via_ctypes("/opt/axon/libaxon_pjrt.so")
        mod = types.ModuleType("antenv.axon_hooks")
        state = {"hook": hook}
        mod.get_axon_ntff_profile_hook = lambda: state["hook"]
        mod.set_axon_ntff_profile_hook = lambda h: state.update(hook=h)
        sys.modules["antenv.axon_hooks"] = mod
        antenv.axon_hooks = mod
    except Exception as e:  # pragma: no cover - best effort
        print(f"ntff hook install failed: {e}", file=sys.stderr)


def _edge_grid(tgt_b):
    """[N, K] edge ids bucketed by target node, padded by duplication."""
    counts = np.bincount(tgt_b, minlength=N)
    if counts.max() > K or counts.min() < 1:
        raise ValueError(f"edge counts per node outside [1, {K}]: "
                         f"min={counts.min()} max={counts.max()}")
    order = np.argsort(tgt_b, kind="stable")
    if (counts == K).all():
        return order.reshape(N, K)
    pos = np.zeros(N + 1, np.int64)
    np.cumsum(counts, out=pos[1:])
    offs = np.minimum(np.arange(K)[None, :], (counts - 1)[:, None])
    return order[pos[:-1, None] + offs]


def kernel(**inputs):
    global LAST_RESULT
    import ml_dtypes
    from concourse.bass_utils import run_bass_kernel_spmd

    wdt = ml_dtypes.bfloat16

    cfg = np.asarray(inputs["cfg_indices_padded"])
    hint_state = np.asarray(inputs["hint_state"], dtype=np.float32)
    node_fts = np.asarray(inputs["node_fts"], dtype=np.float32)
    edge_fts = np.asarray(inputs["edge_fts"], dtype=np.float32)
    edge_W = np.asarray(inputs["edge_W"], dtype=np.float32)
    edge_b = np.asarray(inputs["edge_b"], dtype=np.float32)
    update_W = np.asarray(inputs["update_W"], dtype=np.float32)
    update_b = np.asarray(inputs["update_b"], dtype=np.float32)

    src = np.asarray(cfg[..., 0], dtype=np.int64)
    tgt = np.asarray(cfg[..., 1], dtype=np.int64)

    with_ub = bool(np.any(update_b != 0.0))
    key = ("nc", with_ub)
    if key not in _CACHE:
        _CACHE[key] = _build(with_ub)
    nc = _CACHE[key]

    eW_in = np.ascontiguousarray(edge_W[:, 0].reshape(2, P).T).astype(wdt)
    eb_in = np.full((P, 1), edge_b[0], np.float32)
    ub_in = np.ascontiguousarray(update_b[None, :]).astype(wdt)
    uW_in = update_W.astype(wdt)

    in_maps = []
    for b in range(B):
        hint_b = np.ascontiguousarray(
            hint_state[b].reshape(N, MH)).astype(wdt)
        grid = _edge_grid(tgt[b])             # [N, K]
        srcg = src[b][grid]                   # [N, K]
        for q in range(4):
            g_q = grid[q * NCORE:(q + 1) * NCORE]    # [1024, K]
            s_q = srcg[q * NCORE:(q + 1) * NCORE]
            # gather index order: i = nb*2048 + k*128 + p, wrapped into
            # [16, EC/16] (idx16[r, c] = position c*16+r), tiled to 128 rows.
            gorder = s_q.reshape(NB, P, K).transpose(0, 2, 1)   # [nb, k, p]
            idx16 = np.ascontiguousarray(
                np.tile(gorder.reshape(EC // 16, 16).T, (8, 1))
            ).astype(np.int16)
            # edge-feature column order: j = nb*2048 + p*16 + k (node-major)
            eids = g_q.reshape(NB * P * K)
            efts_t = np.ascontiguousarray(edge_fts[b][eids].T).astype(wdt)
            nf_q = np.ascontiguousarray(
                node_fts[b, q * NCORE:(q + 1) * NCORE].reshape(NCORE, MH)
            ).astype(wdt)
            in_maps.append({
                "hint": hint_b,
                "eftsT": efts_t,
                "idx16": idx16,
                "nf": nf_q,
                "eW": eW_in,
                "eb": eb_in,
                "uW": uW_in,
                "ub": ub_in,
            })

    trace = bool(int(os.environ.get("KERNEL_TRACE", "0")))
    if trace:
        _install_ntff_hook()
    res = run_bass_kernel_spmd(nc, in_maps, core_ids=list(range(N_CORES)),
                               trace=trace)
    if trace:
        LAST_RESULT = res

    out = np.empty((B, N, M, H), np.float32)
    for b in range(B):
        for q in range(4):
            o = np.asarray(res.results[b * 4 + q]["out"], dtype=np.float32)
            out[b, q * NCORE:(q + 1) * NCORE] = o.reshape(NCORE, M, H)
    return out


# revision 13
# speedup vs baseline: 1.2983x; 1.1238x over previous
"""DFA-GNN (max-aggregation message passing) Trainium2 kernel.

Problem (B=2, N=4096, E=65536, M=4, H=256), per batch b:
    coeff[e]  = edge_fts[b,e,:] @ edge_W + edge_b                  # [E]
    agg[n]    = max over edges e with tgt[e]==n of coeff[e] * hint[b, src[e]]
    out[b,n]  = (node_fts[b,n] + agg[n]) @ update_W + update_b     # [M,H] rows

Sharding: 8 cores = 2 batches x 4 target-node quarters (1024 nodes each).
Edges are bucketed by target node on the host (every node has exactly 16
incoming edges with this generator; general counts <=16 are padded by
duplicating an edge, which preserves the max).

Data path is bf16 (harness gate is rel_err < 2e-2; measured ~4.7e-3):
  - gather: one SWDGE dma_gather per 4 edge ranks (512 descriptors per Pool
    instruction, single_packet, round-robin over 4 SWDGE queues) pulls 2KB
    hint rows from HBM at full DMA bandwidth into [128 nodes, 4*1024]
    tiles; 6 tiles in flight bound the pipeline lag (and so the drain
    tail) to ~1.5 blocks,
  - per-edge coefficients via PE matmuls (edge_W stationary, host-transposed
    edge features streaming), sprayed into a [128, 16] f32 per-partition
    layout (coeff for block nb+1 is emitted before block nb's chain),
  - mult+max chain on two accumulators: odd ranks multiply on the Act
    engine (out-of-place, per-partition f32 scale) and TT-max on DVE (2x
    mode); even ranks run the fused scalar_tensor_tensor on DVE (1x; the
    split load-balances DVE ~15.5us vs Act ~14.5us per block),
  - +node_fts as a DVE tensor_tensor add, PE transposes to feature-major,
    update_W matmuls; update_b is folded in as a ones x b rank-1 term
    ONLY when update_b != 0 on the host (saves 4 PE matmuls/block for the
    common zero-bias case),
  - bf16 output, upcast to f32 on the host.

Engine budget per core (8 blocks): DMA ~134us (gather 94 + streams 40,
byte-bound at ~22GB/s/engine x16), DVE ~125us, Act ~115us, Pool ~40us of
real descriptor generation (the rest of its occupancy is ring
backpressure), PE ~96us. Measured 176-200us end-to-end (run-to-run
machine variance ~5-8%); the structural floor is ~20us startup +
8 blocks x ~16.2us (DMA/DVE/Act co-paced) + ~28us pipeline drain.

Things measured NOT to work on this hardware, kept here so future
sessions skip them (a full session was burned re-learning this list):
  - INT8 gather (per-node row scales folded into the coeff on host, sim
    rel err 7.8e-3) halves gather DMA to 68us and passes accuracy on hw
    (9.2e-3) but REGRESSES end-to-end to 229us: 1-byte operands disable
    the DVE 2x/4x perf modes and slow Act ~1.4x (int8 Act mul 1500ns,
    int8 STT 1500ns, int8 tensor_scalar 2464ns+ for [128,1024]); the
    elementwise side becomes the bottleneck. fp8_e4m3 hint fails
    accuracy (3.0e-2); fp8_e3m4 edge features (edge_W pre-scaled x64 to
    dodge subnormals) sim at 1.8e-2 -- too close to the 2e-2 gate.
  - Wide-tile restructure (independent products into [128,4096] tiles +
    wide TT max tree at 2306ns per 4 maxes + DMA xbar transpose
    replacing PE transposes + deferred o-copy) has better theoretical
    engine sums (Act ~13us, DVE ~13us per block) but pipelines WORSE
    than this fine-grained chain: engines sit ~50% occupied waiting on
    gathers / each other and it lands at 222-256us even after fixing
    queue head-of-line orders. Fine-grained rank-by-rank emission is
    what keeps all five queues busy here.
  - DMA cannot read PSUM (coeff spray must bounce through SBUF via Act).
  - Matmul PSUM outputs must start at partition 0/32/64; Activation
    operands cannot use strided partition steps.
  - Act engine: (N+352)cyc/1.2GHz, no 2x mode; "dtype-independent" is
    false for int8 (1.33x slower). DVE at ~0.96GHz: TT max is 2x_1p
    only (~825ns/1024 out-of-place, ~680 in-place); STT is 1x-only;
    tensor_scalar reaches 4x (~400ns) only with 2-byte dtypes, all-SBUF
    operands and low gather-write contention (else degrades to 1x-2.4x).
  - multi-index indirect DMA corrupts data (one index per partition per
    SWDGE trigger); ap_gather executes asynchronously at ~61us per
    2048x16B gather (9x its cost model); Pool/GpSimd cannot run
    tensor_tensor / scalar_tensor_tensor (walrus engine check); bf16
    PSUM cannot accumulate, and PE transpose output dtype must match
    lhsT, so +node_fts cannot fold into the transpose in bf16; PE matmul
    rejects int8 (fp8e3/e4/e5 ok); PE down-clocks when idle (first
    matmuls after a gap run ~2x slow); in-place DVE/Act elementwise ops
    lose their perf mode (~15% penalty).
"""

import os
import sys

import numpy as np

for _p in ("/opt/trn_rl_repo", "/root/.axon_site/_ro/trn_rl_repo"):
    if os.path.isdir(_p) and _p not in sys.path:
        sys.path.insert(0, _p)

B, N, E, M, H = 2, 4096, 65536, 4, 256
MH = M * H            # 1024
P = 128               # partitions
K = 16                # edges per node (E // N)
NCORE = N // 4        # nodes per core (1024)
NB = NCORE // P       # node blocks per core (8)
EC = NCORE * K        # edges per core (16384)
ECB = P * K           # edges per block (2048)
KH = K // 4           # edge ranks per gather tile (4)
GT_BUFS = int(os.environ.get("KERNEL_GT_BUFS", "6"))
N_CORES = 8
N_SWDGE_Q = int(os.environ.get("KERNEL_SWDGE_Q", "4"))

_CACHE = {}

# Set by kernel() when KERNEL_TRACE=1: BassKernelResults of the last run.
LAST_RESULT = None


def _build(with_ub: bool):
    from concourse import bass, bacc, mybir, tile

    f32 = mybir.dt.float32
    i16 = mybir.dt.int16
    bf16 = mybir.dt.bfloat16

    nc = bacc.Bacc("TRN2", target_bir_lowering=False, debug=False,
                   num_devices=N_CORES, num_swdge_queues=N_SWDGE_Q)

    hint = nc.dram_tensor("hint", [N, MH], bf16, kind="ExternalInput")
    eftsT = nc.dram_tensor("eftsT", [H, EC], bf16, kind="ExternalInput")
    idx_d = nc.dram_tensor("idx16", [P, EC // 16], i16, kind="ExternalInput")
    nf_d = nc.dram_tensor("nf", [NCORE, MH], bf16, kind="ExternalInput")
    eW_d = nc.dram_tensor("eW", [P, 2], bf16, kind="ExternalInput")
    eb_d = nc.dram_tensor("eb", [P, 1], f32, kind="ExternalInput")
    uW_d = nc.dram_tensor("uW", [H, H], bf16, kind="ExternalInput")
    ub_d = nc.dram_tensor("ub", [1, H], bf16, kind="ExternalInput")
    out_d = nc.dram_tensor("out", [NCORE, MH], bf16, kind="ExternalOutput")

    with tile.TileContext(nc) as tc:
        from concourse.mybir import AluOpType as alu

        with (
            tc.tile_pool(name="const", bufs=1) as cpool,
            tc.tile_pool(name="efts", bufs=2) as epool,
            tc.tile_pool(name="gt", bufs=GT_BUFS) as gpool,
            tc.tile_pool(name="sc", bufs=3) as scpool,
            tc.tile_pool(name="work", bufs=2) as wpool,
            tc.tile_pool(name="ps_coeff", bufs=2, space="PSUM") as ps_coeff,
            tc.tile_pool(name="ps_xt", bufs=2, space="PSUM") as ps_xt,
            tc.tile_pool(name="ps_out", bufs=1, space="PSUM") as ps_out,
        ):
            from concourse.masks import make_identity

            # idx16 rides the Act-engine HWDGE queue: the sync queue floods
            # with 1MB of edge features at startup and would delay the first
            # gather's index table by ~10us.
            idx_t = cpool.tile([P, EC // 16], i16)
            nc.scalar.dma_start(out=idx_t[:], in_=idx_d[:])
            ident = cpool.tile([P, P], bf16)
            make_identity(nc, ident[:])
            eW = cpool.tile([P, 2], bf16)
            nc.sync.dma_start(out=eW[:], in_=eW_d[:])
            eb = cpool.tile([P, 1], f32)
            nc.sync.dma_start(out=eb[:], in_=eb_d[:])
            uW0 = cpool.tile([P, H], bf16)
            uW1 = cpool.tile([P, H], bf16)
            nc.sync.dma_start(out=uW0[:], in_=uW_d[0:P, :])
            nc.sync.dma_start(out=uW1[:], in_=uW_d[P:2 * P, :])
            if with_ub:
                ub_row = cpool.tile([1, H], bf16)
                nc.sync.dma_start(out=ub_row[:], in_=ub_d[:])
                ones1 = cpool.tile([1, P], bf16)
                nc.vector.memset(ones1[:], 1.0)

            coeffs = [None] * NB

            def emit_coeff(nb):
                # eftsT columns node-major in the block:
                # col nb*2048 + p*16 + k -> edge rank k of node p.
                efts0 = epool.tile([P, ECB], bf16, tag="efts0")
                efts1 = epool.tile([P, ECB], bf16, tag="efts1")
                nc.sync.dma_start(out=efts0[:],
                                  in_=eftsT[0:P, nb * ECB:(nb + 1) * ECB])
                nc.sync.dma_start(out=efts1[:],
                                  in_=eftsT[P:2 * P, nb * ECB:(nb + 1) * ECB])
                co_ps = ps_coeff.tile([128, 1024], f32, tag="co_ps",
                                      space="PSUM")
                for c in range(4):
                    pp, ff = (c % 2) * 64, (c // 2) * 512
                    nc.tensor.matmul(co_ps[pp:pp + 1, ff:ff + 512],
                                     lhsT=eW[:, 0:1],
                                     rhs=efts0[:, c * 512:(c + 1) * 512],
                                     start=True, stop=False)
                    nc.tensor.matmul(co_ps[pp:pp + 1, ff:ff + 512],
                                     lhsT=eW[:, 1:2],
                                     rhs=efts1[:, c * 512:(c + 1) * 512],
                                     start=False, stop=True)
                co_row = wpool.tile([P, 512], f32, tag="co_row")
                for c in range(4):
                    pp, ff = (c % 2) * 64, (c // 2) * 512
                    nc.scalar.add(co_row[c * 32:c * 32 + 1, :],
                                  co_ps[pp:pp + 1, ff:ff + 512], eb[0:1, 0:1])
                coeff = wpool.tile([P, K], f32, tag="coeff")
                for c in range(4):
                    nc.sync.dma_start(
                        out=coeff[c * 32:(c + 1) * 32, :],
                        in_=co_row[c * 32:c * 32 + 1, :].rearrange(
                            "c (p k) -> c p k", k=K))
                coeffs[nb] = coeff

            emit_coeff(0)
            # one shared num_idxs register for every dma_gather (all 512)
            nidx_reg = nc.gpsimd.to_reg(P * KH)
            for nb in range(NB):
                # ---- gather: one SWDGE dma_gather per 4 edge ranks ----
                gts = []
                for h in range(K // KH):
                    gt = gpool.tile([P, KH * MH], bf16, tag="gt")
                    c0 = (nb * ECB + h * P * KH) // 16
                    nc.gpsimd.dma_gather(
                        gt[:].rearrange("p (g e) -> p g e", e=MH),
                        hint[:],
                        idx_t[:, c0:c0 + P * KH // 16],
                        P * KH, nidx_reg, MH,
                        queue_num=(nb * (K // KH) + h) % N_SWDGE_Q,
                    )
                    gts.append(gt)
                nf = wpool.tile([P, MH], bf16, tag="nf")
                nc.scalar.dma_start(out=nf[:], in_=nf_d[nb * P:(nb + 1) * P, :])
                if nb + 1 < NB:
                    emit_coeff(nb + 1)

                # ---- mult+max chain, two accumulators (even/odd rank) ---
                # even ranks >=2 run fused STT on DVE; odd ranks multiply
                # out-of-place on Act, then TT-max on DVE (2x mode).
                coeff = coeffs[nb]
                acc_a = wpool.tile([P, MH], bf16, tag="acc_a")
                acc_b = wpool.tile([P, MH], bf16, tag="acc_b")
                accs = [acc_a, acc_b]
                for h in range(K // KH):
                    gt = gts[h]
                    for j in range(KH):
                        k = h * KH + j
                        src = gt[:, j * MH:(j + 1) * MH]
                        sc = coeff[:, k:k + 1]
                        acc = accs[k % 2]
                        if k == 0:
                            nc.vector.tensor_scalar(
                                out=acc[:], in0=src, scalar1=sc,
                                scalar2=None, op0=alu.mult)
                        elif k == 1:
                            nc.scalar.mul(acc[:], src, sc)
                        elif k % 2 == 1:
                            sct = scpool.tile([P, MH], bf16, tag="sct")
                            nc.scalar.mul(sct[:], src, sc)
                            nc.vector.tensor_tensor(out=acc[:], in0=sct[:],
                                                    in1=acc[:], op=alu.max)
                        else:
                            nc.vector.scalar_tensor_tensor(
                                out=acc[:], in0=src, scalar=sc,
                                in1=acc[:], op0=alu.mult, op1=alu.max)

                # ---- combine, +node_fts, transpose, update matmuls ----
                acc = acc_a
                nc.vector.tensor_tensor(out=acc[:], in0=acc_b[:], in1=acc[:],
                                        op=alu.max)
                nc.vector.tensor_tensor(out=acc[:], in0=acc[:], in1=nf[:],
                                        op=alu.add)
                xt_ps = ps_xt.tile([P, MH], bf16, tag="xt_ps", space="PSUM")
                for c in range(MH // P):
                    nc.tensor.matmul(xt_ps[:, c * P:(c + 1) * P],
                                     lhsT=acc[:, c * P:(c + 1) * P],
                                     rhs=ident[:], is_transpose=True,
                                     start=True, stop=True)
                xt = wpool.tile([P, MH], bf16, tag="xt")
                nc.scalar.copy(xt[:], xt_ps[:])

                o_ps = ps_out.tile([P, MH], f32, tag="o_ps", space="PSUM")
                for m in range(M):
                    nc.tensor.matmul(o_ps[:, m * H:(m + 1) * H],
                                     lhsT=xt[:, (2 * m) * P:(2 * m + 1) * P],
                                     rhs=uW0[:], start=True, stop=False)
                    nc.tensor.matmul(o_ps[:, m * H:(m + 1) * H],
                                     lhsT=xt[:, (2 * m + 1) * P:(2 * m + 2) * P],
                                     rhs=uW1[:], start=False,
                                     stop=not with_ub)
                    if with_ub:
                        nc.tensor.matmul(o_ps[:, m * H:(m + 1) * H],
                                         lhsT=ones1[0:1, :], rhs=ub_row[0:1, :],
                                         start=False, stop=True)
                o = wpool.tile([P, MH], bf16, tag="o")
                nc.scalar.copy(o[:], o_ps[:])
                nc.scalar.dma_start(out=out_d[nb * P:(nb + 1) * P, :], in_=o[:])

    nc.compile()
    return nc


def _install_ntff_hook():
    """Register the axon NTFF profiling hook if this image's antenv lacks it.

    Mirrors what trn_boot does when ``antenv.axon_hooks`` exists. Safe no-op
    on failure — tracing is skipped, execution still works.
    """
    import types

    try:
        import antenv.axon_hooks  # noqa: F401
        return
    except ImportError:
        pass
    try:
        import antenv
        from trn_agent_boot.trn_boot import _ntff_profile_via_ctypes

        hook = _ntff_profile_via_ctypes("/opt/axon/libaxon_pjrt.so")
        mod = types.ModuleType("antenv.axon_hooks")
        state = {"hook": hook}
        mod.get_axon_ntff_profile_hook = lambda: state["hook"]
        mod.set_axon_ntff_profile_hook = lambda h: state.update(hook=h)
        sys.modules["antenv.axon_hooks"] = mod
        antenv.axon_hooks = mod
    except Exception as e:  # pragma: no cover - best effort
        print(f"ntff hook install failed: {e}", file=sys.stderr)


def _edge_grid(tgt_b):
    """[N, K] edge ids bucketed by target node, padded by duplication."""
    counts = np.bincount(tgt_b, minlength=N)
    if counts.max() > K or counts.min() < 1:
        raise ValueError(f"edge counts per node outside [1, {K}]: "
                         f"min={counts.min()} max={counts.max()}")
    order = np.argsort(tgt_b, kind="stable")
    if (counts == K).all():
        return order.reshape(N, K)
    pos = np.zeros(N + 1, np.int64)
    np.cumsum(counts, out=pos[1:])
    offs = np.minimum(np.arange(K)[None, :], (counts - 1)[:, None])
    return order[pos[:-1, None] + offs]


def kernel(**inputs):
    global LAST_RESULT
    import ml_dtypes
    from concourse.bass_utils import run_bass_kernel_spmd

    wdt = ml_dtypes.bfloat16

    cfg = np.asarray(inputs["cfg_indices_padded"])
    hint_state = np.asarray(inputs["hint_state"], dtype=np.float32)
    node_fts = np.asarray(inputs["node_fts"], dtype=np.float32)
    edge_fts = np.asarray(inputs["edge_fts"], dtype=np.float32)
    edge_W = np.asarray(inputs["edge_W"], dtype=np.float32)
    edge_b = np.asarray(inputs["edge_b"], dtype=np.float32)
    update_W = np.asarray(inputs["update_W"], dtype=np.float32)
    update_b = np.asarray(inputs["update_b"], dtype=np.float32)

    src = np.asarray(cfg[..., 0], dtype=np.int64)
    tgt = np.asarray(cfg[..., 1], dtype=np.int64)

    with_ub = bool(np.any(update_b != 0.0))
    key = ("nc", with_ub)
    if key not in _CACHE:
        _CACHE[key] = _build(with_ub)
    nc = _CACHE[key]

    eW_in = np.ascontiguousarray(edge_W[:, 0].reshape(2, P).T).astype(wdt)
    eb_in = np.full((P, 1), edge_b[0], np.float32)
    ub_in = np.ascontiguousarray(update_b[None, :]).astype(wdt)
    uW_in = update_W.astype(wdt)

    in_maps = []
    for b in range(B):
        hint_b = np.ascontiguousarray(
            hint_state[b].reshape(N, MH)).astype(wdt)
        grid = _edge_grid(tgt[b])             # [N, K]
        srcg = src[b][grid]                   # [N, K]
        for q in range(4):
            g_q = grid[q * NCORE:(q + 1) * NCORE]    # [1024, K]
            s_q = srcg[q * NCORE:(q + 1) * NCORE]
            # gather index order: i = nb*2048 + k*128 + p, wrapped into
            # [16, EC/16] (idx16[r, c] = position c*16+r), tiled to 128 rows.
            gorder = s_q.reshape(NB, P, K).transpose(0, 2, 1)   # [nb, k, p]
            idx16 = np.ascontiguousarray(
                np.tile(gorder.reshape(EC // 16, 16).T, (8, 1))
            ).astype(np.int16)
            # edge-feature column order: j = nb*2048 + p*16 + k (node-major)
            eids = g_q.reshape(NB * P * K)
            efts_t = np.ascontiguousarray(edge_fts[b][eids].T).astype(wdt)
            nf_q = np.ascontiguousarray(
                node_fts[b, q * NCORE:(q + 1) * NCORE].reshape(NCORE, MH)
            ).astype(wdt)
            in_maps.append({
                "hint": hint_b,
                "eftsT": efts_t,
                "idx16": idx16,
                "nf": nf_q,
                "eW": eW_in,
                "eb": eb_in,
                "uW": uW_in,
                "ub": ub_in,
            })

    trace = bool(int(os.environ.get("KERNEL_TRACE", "0")))
    if trace:
        _install_ntff_hook()
    res = run_bass_kernel_spmd(nc, in_maps, core_ids=list(range(N_CORES)),
                               trace=trace)
    if trace:
        LAST_RESULT = res

    out = np.empty((B, N, M, H), np.float32)
    for b in range(B):
        for q in range(4):
            o = np.asarray(res.results[b * 4 + q]["out"], dtype=np.float32)
            out[b, q * NCORE:(q + 1) * NCORE] = o.reshape(NCORE, M, H)
    return out
